# revision 1
# baseline (speedup 1.0000x reference)
"""DigitCaps dynamic-routing kernel for 8 TRN2 NeuronCores.

Problem (hardcoded): x [256,1152,8] f32, W [1,1152,10,16,8] f32, 3 routing
iterations -> v [256,10,16,1] f32.

Strategy: shard the R=1152 routes 8-ways (144 per core), keep the full batch
B=256 on every core. u_hat is never materialized; each routing iteration
streams W through the TensorEngine:
  s_c[o,b]   = sum_{(r,i)} Ws_c[(r,i),o] * (en_c[r,b] * x[(r,i),b])   (PE)
  (AllReduce s over the 8 R-shards, squash -> v on every core)
  M_c[b,(r,i)] = sum_o v_c[b,o] * WoT_c[o,(r,i)]                      (PE)
  a_c[b,r]   = sum_i x[b,(r,i)] * M_c[b,(r,i)]                        (DVE)
Logits/softmax stay in [partition=b%128, free=(bh,c,r)] layout; the e->eT
transpose for the s-matmul runs on the PE with a replicate-by-8 DMA.
All data stays f32: the routing argmax is chaotic under bf16 rounding
(measured 5e-2 output error from bf16 W/x vs 4e-6 for f32).
"""

import sys

if "/opt/trn_rl_repo" not in sys.path:
    sys.path.insert(0, "/opt/trn_rl_repo")

import numpy as np

import concourse.bass as bass
import concourse.tile as tile
from concourse import bacc, mybir
from concourse.bass_utils import run_bass_kernel_spmd
from concourse.masks import make_identity

F32 = mybir.dt.float32
BF16 = mybir.dt.bfloat16

NCORES = 8
B, R, C, O, I = 256, 1152, 10, 16, 8
RL = R // NCORES          # 144 routes per core
RI = RL * I               # 1152 (r,i) rows per core
NT = RI // 128            # 9 K-chunks of 128
CO = C * O                # 160
BH = B // 128             # 2 batch half-tiles

AP = bass.AP


def _insert_bcast(base, pos, count):
    """Insert a step-0 (broadcast) free dim into an existing AP at index pos."""
    dims = list(base.ap)
    dims.insert(pos, [0, count])
    return AP(tensor=base.tensor, offset=base.offset, ap=dims)


def build_kernel(n_iters: int, reps: int = 1, collectives: bool = True):
    nc = bacc.Bacc("TRN2", target_bir_lowering=False, debug=False,
                   num_devices=NCORES)

    xt_in = nc.dram_tensor("xt", [128, NT, B], F32, kind="ExternalInput")
    xb_in = nc.dram_tensor("xb", [128, BH, RI], F32, kind="ExternalInput")
    ws = nc.dram_tensor("ws", [128, NT, CO], F32, kind="ExternalInput")
    wot = nc.dram_tensor("wot", [16, C, RI], F32, kind="ExternalInput")
    out = nc.dram_tensor("out", [B, CO], F32, kind="ExternalOutput")

    with tile.TileContext(nc) as tc:
        with (
            tc.tile_pool(name="stat", bufs=1) as stat,
            tc.tile_pool(name="work", bufs=2) as work,
            tc.tile_pool(name="sm", bufs=1) as smp,
            tc.tile_pool(name="ent", bufs=4) as entp,
            tc.tile_pool(name="ytp", bufs=2) as ytp,
            tc.tile_pool(name="mtp", bufs=4) as mtp,
            tc.tile_pool(name="dram", bufs=2, space="DRAM") as dram,
            tc.tile_pool(name="ps_mp", bufs=3, space="PSUM") as ps_mp,
            tc.tile_pool(name="ps_ep", bufs=3, space="PSUM") as ps_ep,
            tc.tile_pool(name="ps_sp", bufs=2, space="PSUM") as ps_tr,
        ):
            # ---- static SBUF tensors ----
            XT = stat.tile([128, NT, B], F32)        # x^T  [(r,i)%128, t, b]
            XB = stat.tile([128, BH, RI], F32)       # x    [b%128, bh, (r,i)]
            WS = stat.tile([128, NT, CO], F32)       # W as lhsT for s-matmul
            WOT = stat.tile([16, C, RI], F32)        # W^T as rhs for M-matmul
            IDENT = stat.tile([128, 128], F32)
            nc.sync.dma_start(out=XT, in_=xt_in[:])
            nc.sync.dma_start(out=XB, in_=xb_in[:])
            nc.sync.dma_start(out=WS, in_=ws[:])
            nc.sync.dma_start(out=WOT, in_=wot[:])
            make_identity(nc, IDENT[:, :])

            # logits b_ij, layout [p=b%128, (bh, c, r)]
            blog = stat.tile([128, BH, C, RL], F32)
            nc.vector.memset(blog, 0.0)

            # v (squashed capsule outputs), [p=b%128, (bh, co)]
            vsb = stat.tile([128, BH, CO], F32)

            def s0_matmul():
                """s0 = 0.1 * sum_r u_hat  ->  psum [co, b] (two tiles)."""
                p1 = ps_ep.tile([128, B], F32, tag="ep")
                p2 = ps_ep.tile([32, B], F32, tag="ep")
                for t in range(NT):
                    xcol = XT[:, t, :]             # [128, 256]
                    nc.tensor.matmul(p1, WS[:, t, 0:128], xcol,
                                     start=(t == 0), stop=(t == NT - 1))
                    nc.tensor.matmul(p2, WS[:, t, 128:160], xcol,
                                     start=(t == 0), stop=(t == NT - 1))
                return p1, p2

            def dma_psum_to_bounce(ptile, nrows, co0, bounce):
                # psum [nrows(co), 256(b)] -> DRAM bounce [256, 160] at col co0
                sb = work.tile([nrows, B], F32, tag=f"sdrain{nrows}")
                nc.scalar.copy(sb[:, :], ptile[0:nrows, :])
                dst = bounce[:, co0:co0 + nrows].rearrange("b co -> co b")
                nc.sync.dma_start(out=dst, in_=sb[:, :])

            def allreduce_s(writes):
                """writes: list of (ptile, nrows, co0). Returns bounce_out."""
                b_in = dram.tile([B, CO], F32, tag="arin")
                b_out = dram.tile([B, CO], F32, tag="arout")
                for ptile, nrows, co0 in writes:
                    dma_psum_to_bounce(ptile, nrows, co0, b_in)
                if collectives:
                    nc.gpsimd.collective_compute(
                        "AllReduce",
                        mybir.AluOpType.add,
                        replica_groups=[list(range(NCORES))],
                        ins=[b_in[:].opt()],
                        outs=[b_out[:].opt()],
                    )
                else:
                    nc.sync.dma_start(out=b_out[:], in_=b_in[:])
                return b_out

            def squash(b_out, scale):
                """load s from bounce, v = s*|s|/(1+s^2) (optionally s*=scale)"""
                s = work.tile([128, BH, CO], F32, tag="sq_s")
                src = b_out[:].rearrange("(bh p) co -> p bh co", p=128)
                nc.sync.dma_start(out=s, in_=src)
                sf = s[:, :, :]
                sq = work.tile([128, BH, CO], F32, tag="sq_sq")
                ab = work.tile([128, BH, CO], F32, tag="sq_ab")
                den = work.tile([128, BH, CO], F32, tag="sq_den")
                if scale != 1.0:
                    nc.scalar.mul(sf, sf, scale)
                nc.scalar.square(sq[:, :, :], sf)
                nc.scalar.sqrt(ab[:, :, :], sq[:, :, :])
                nc.vector.tensor_scalar_add(den[:, :, :], sq[:, :, :], 1.0)
                nc.vector.reciprocal(den[:, :, :], den[:, :, :])
                nc.vector.tensor_mul(ab[:, :, :], ab[:, :, :], den[:, :, :])
                nc.vector.tensor_mul(vsb[:, :, :], ab[:, :, :], sf)

            def v_transpose():
                """vsb [p=b%128,(bh,co)] f32 -> vT f32 [16(o), c, b]."""
                vt = work.tile([16, C, B], F32, tag="vt")
                for c in range(C):
                    ptc = ps_tr.tile([16, B], F32, tag="m")
                    for bh in range(BH):
                        nc.tensor.matmul(ptc[:, bh * 128:(bh + 1) * 128],
                                         vsb[:, bh, c * 16:(c + 1) * 16],
                                         IDENT[:, :],
                                         start=True, stop=True,
                                         is_transpose=True)
                    nc.scalar.copy(vt[:, c, :], ptc[:, :])
                return vt

            def a_phase(vt, first):
                """blog (+)= a, a_c[b,r] = sum_i x*M, M = v_c @ WoT_c."""
                ar = smp.tile([128, BH, C, RL], F32, tag="ared")
                H = RI // 3
                for c in range(C):
                    for bh in range(BH):
                        lhs = vt[:, c, bh * 128:(bh + 1) * 128]
                        mt = mtp.tile([128, RI], F32, tag="mtmp")
                        for h in range(3):
                            mp = ps_mp.tile([128, H], F32, tag="mpsum")
                            nc.tensor.matmul(mp[:, :], lhs,
                                             WOT[:, c, h * H:(h + 1) * H],
                                             start=True, stop=True)
                            nc.scalar.copy(mt[:, h * H:(h + 1) * H], mp[:, :])
                        eng = nc.vector if (c % 3) else nc.gpsimd
                        eng.tensor_mul(mt[:, :], mt[:, :], XB[:, bh, :])
                        tv = mt[:, :].rearrange("p (r i) -> p r i", i=I)
                        nc.vector.tensor_reduce(ar[:, bh, c, :], tv,
                                                axis=mybir.AxisListType.X,
                                                op=mybir.AluOpType.add)
                if first:
                    nc.vector.tensor_copy(blog[:, :, :, :], ar[:, :, :, :])
                else:
                    nc.vector.tensor_add(blog[:, :, :, :], blog[:, :, :, :],
                                         ar[:, :, :, :])

            def s_phase():
                """softmax(blog) -> en -> enT-rep -> y -> s psum tiles."""
                # shift logits by max over c (persistent; softmax-invariant)
                mx = smp.tile([128, BH, RL], F32, tag="z")
                bv = blog[:, :, :, :].rearrange("p bh c r -> p bh r c")
                nc.vector.tensor_reduce(mx[:, :, :], bv,
                                        axis=mybir.AxisListType.X,
                                        op=mybir.AluOpType.max)
                mrep = smp.tile([128, BH, C, RL], F32, tag="zr")
                nc.gpsimd.tensor_copy(mrep[:, :, :, :],
                                      _insert_bcast(mx[:, :, :], 2, C))
                nc.gpsimd.tensor_sub(blog[:, :, :, :], blog[:, :, :, :],
                                      mrep[:, :, :, :])
                e = smp.tile([128, BH, C, RL], F32, tag="e")
                nc.scalar.activation(e[:, :, :, :], blog[:, :, :, :],
                                     mybir.ActivationFunctionType.Exp)
                z = smp.tile([128, BH, RL], F32, tag="z")
                ev = e[:, :, :, :].rearrange("p bh c r -> p bh r c")
                nc.vector.tensor_reduce(z[:, :, :], ev,
                                        axis=mybir.AxisListType.X,
                                        op=mybir.AluOpType.add)
                nc.vector.reciprocal(z[:, :, :], z[:, :, :])
                zrep = smp.tile([128, BH, C, RL], F32, tag="zr")
                nc.gpsimd.tensor_copy(zrep[:, :, :, :],
                                      _insert_bcast(z[:, :, :], 2, C))
                en = e
                nc.vector.tensor_mul(en[:, :, :, :], e[:, :, :, :],
                                     zrep[:, :, :, :])

                writes = []
                for c in range(C):
                    ep1 = ps_ep.tile([128, B], F32, tag="ep")
                    ep2 = ps_ep.tile([16, B], F32, tag="ep")
                    for bh in range(BH):
                        nc.tensor.matmul(ep1[:, bh * 128:(bh + 1) * 128],
                                         en[:, bh, c, 0:128], IDENT[:, :],
                                         start=True, stop=True,
                                         is_transpose=True)
                        nc.tensor.matmul(ep2[:, bh * 128:(bh + 1) * 128],
                                         en[:, bh, c, 128:RL], IDENT[:, :],
                                         start=True, stop=True,
                                         is_transpose=True)
                    et1 = entp.tile([128, B], BF16, tag="et1")
                    et2 = entp.tile([16, B], BF16, tag="et2")
                    nc.scalar.copy(et1[:, :], ep1[:, :])
                    nc.scalar.copy(et2[:, :], ep2[:, :])
                    etr = ytp.tile([128, NT, B], BF16, tag="etr")
                    for t in range(NT):
                        if t < 8:
                            base = et1[16 * t:16 * t + 16, :]
                        else:
                            base = et2[0:16, :]
                        src = _insert_bcast(base, 1, I)
                        qeng = nc.sync if (t % 2 == 0) else nc.scalar
                        qeng.dma_start(out=etr[:, t, :], in_=src)
                    ytc = ytp.tile([128, NT, B], F32, tag="ytc")
                    nc.vector.tensor_mul(ytc[:, :, :], etr[:, :, :],
                                         XT[:, :, :])
                    sp = ps_tr.tile([16, B], F32, tag="m")
                    for t in range(NT):
                        nc.tensor.matmul(sp, WS[:, t, c * 16:(c + 1) * 16],
                                         ytc[:, t, :],
                                         start=(t == 0), stop=(t == NT - 1))
                    writes.append((sp, 16, c * 16))
                return writes

            # ---------------- routing ----------------
            for _rep in range(reps):
                p1, p2 = s0_matmul()
                bout = allreduce_s([(p1, 128, 0), (p2, 32, 128)])
                squash(bout, 0.1)
                for it in range(1, n_iters):
                    vt = v_transpose()
                    a_phase(vt, first=(it == 1))
                    writes = s_phase()
                    bout = allreduce_s(writes)
                    squash(bout, 1.0)

            dst = out[:].rearrange("(bh p) co -> p bh co", p=128)
            nc.sync.dma_start(out=dst, in_=vsb[:, :, :])

    nc.compile()
    return nc


def prep_inputs(x: np.ndarray, W: np.ndarray):
    """Host-side layout prep. Returns per-core input dicts."""
    W = W[0]  # [R, C, O, I]
    in_maps = []
    for k in range(NCORES):
        rs = slice(k * RL, (k + 1) * RL)
        xk = np.ascontiguousarray(x[:, rs, :])      # [B, RL, I]
        wk = np.ascontiguousarray(W[rs])            # [RL, C, O, I]
        xt = np.transpose(xk, (1, 2, 0)).reshape(NT, 128, B)
        xt = np.transpose(xt, (1, 0, 2))            # [128, NT, B]
        xb = xk.reshape(BH, 128, RI)
        xb = np.transpose(xb, (1, 0, 2))            # [128, BH, RI]
        # ws[p, t, c*16+o] = W[16t + p//8, c, o, p%8]
        wsk = np.transpose(wk.reshape(NT, 16, C, O, I), (0, 1, 4, 2, 3))
        wsk = wsk.reshape(NT, 128, CO)
        wsk = np.transpose(wsk, (1, 0, 2))          # [128, NT, CO]
        # wot[o, c, r*8+i] = W[r, c, o, i]
        wotk = np.transpose(wk, (2, 1, 0, 3)).reshape(O, C, RI)
        f32 = np.float32
        in_maps.append({
            "xt": np.ascontiguousarray(xt).astype(f32),
            "xb": np.ascontiguousarray(xb).astype(f32),
            "ws": np.ascontiguousarray(wsk).astype(f32),
            "wot": np.ascontiguousarray(wotk).astype(f32),
        })
    return in_maps


_CACHE = {}


def _get_nc(n_iters: int):
    if n_iters not in _CACHE:
        _CACHE[n_iters] = build_kernel(n_iters)
    return _CACHE[n_iters]


def kernel(x, W, num_iterations, _trace=False):
    n = int(num_iterations)
    assert n >= 1
    nc = _get_nc(n)
    in_maps = prep_inputs(np.asarray(x, dtype=np.float32),
                          np.asarray(W, dtype=np.float32))
    res = run_bass_kernel_spmd(nc, in_maps, list(range(NCORES)),
                               trace=_trace)
    v = res.results[0]["out"].reshape(B, C, O, 1).astype(np.float32)
    kernel.last_results = res
    return v



# revision 17
# speedup vs baseline: 2.1819x; 2.1819x over previous
"""DigitCaps dynamic-routing kernel for 8 TRN2 NeuronCores (v2).

Problem (hardcoded): x [256,1152,8] f32, W [1,1152,10,16,8] f32, 3 routing
iterations -> v [256,10,16,1] f32.

Strategy: shard the R=1152 routes 8-ways (144 per core), full batch B=256 on
every core. u_hat is never materialized; each iteration streams W through the
TensorEngine:
  s_c[o,b]   = sum_{(r,i)} Ws_c[(r,i),(c,o)] * (en_c[r,b] * x[(r,i),b])  (PE)
  (AllReduce s over the 8 R-shards in [CO,B] layout, squash -> v)
  M_c[b,(r,i)] = sum_o v_c[o,b] * WoT_c[o,(r,i)]                         (PE)
  a_c[b,r]   = sum_i x[b,(r,i)] * M_c[b,(r,i)]                           (DVE)

v2 changes vs baseline:
- AllReduce bounce kept in [CO,B] layout (contiguous descriptors; the old
  transposed write emitted ~41k 4-byte descriptors / 100us per phase).
- v lives in [co,b]; v^T for the M-matmul is 10 small realign DMAs; the
  whole v_transpose PE phase is gone. Output transposed once at the end.
- softmax without max-shift (logits are bounded ~+-30, exp is f32-safe);
  z-reciprocal applied via step-0 broadcast AP (no zrep materialization).
- en in bf16 (measured end-to-end impact ~5e-3); en-transpose via normal
  matmul against a bf16 identity (1cyc/row vs 4 for fp32 LOW_HIGH).
- M-path (a-phase) in bf16, s-path f32 except the last iteration (bf16);
  measured combined rel err ~6e-3 vs the 2e-2 gate.
- dummy 4-byte AllReduce issued first to absorb the cc entry barrier /
  ncfw warmup under the input load.
- engine spread: psum drains and big elementwise ops split across
  Scalar/Vector/GpSimd so no single engine serializes; GpSimd kept free
  near collective triggers.
"""

import sys

if "/opt/trn_rl_repo" not in sys.path:
    sys.path.insert(0, "/opt/trn_rl_repo")

import numpy as np
import ml_dtypes

import concourse.bass as bass
import concourse.tile as tile
from concourse import bacc, mybir
from concourse.bass_utils import run_bass_kernel_spmd
from concourse.masks import make_identity

F32 = mybir.dt.float32
BF16 = mybir.dt.bfloat16

NCORES = 8
B, R, C, O, I = 256, 1152, 10, 16, 8
RL = R // NCORES          # 144 routes per core
RI = RL * I               # 1152 (r,i) rows per core
NT = RI // 128            # 9 K-chunks of 128
CO = C * O                # 160
BH = B // 128             # 2 batch half-tiles
H = RI // 3               # 384: M-matmul free chunk

AP = bass.AP


def _insert_bcast(base, pos, count):
    """Insert a step-0 (broadcast) free dim into an existing AP at index pos."""
    dims = list(base.ap)
    dims.insert(pos, [0, count])
    return AP(tensor=base.tensor, offset=base.offset, ap=dims)


def build_kernel(n_iters: int, collectives: bool = True):
    nc = bacc.Bacc("TRN2", target_bir_lowering=False, debug=False,
                   num_devices=NCORES)

    xt_in = nc.dram_tensor("xt", [128, NT, B], F32, kind="ExternalInput")
    xb_in = nc.dram_tensor("xb", [128, BH, RI], F32, kind="ExternalInput")
    ws_in = nc.dram_tensor("ws", [128, NT, CO], F32, kind="ExternalInput")
    wot_in = nc.dram_tensor("wot", [16, C, RI], BF16, kind="ExternalInput")
    out = nc.dram_tensor("out", [B, CO], F32, kind="ExternalOutput")

    with tile.TileContext(nc) as tc:
        with (
            tc.tile_pool(name="stat", bufs=1) as stat,
            tc.tile_pool(name="work", bufs=2) as work,
            tc.tile_pool(name="sm", bufs=1) as smp,
            tc.tile_pool(name="ent", bufs=3) as entp,
            tc.tile_pool(name="ytp", bufs=2) as ytp,
            tc.tile_pool(name="mtp", bufs=3) as mtp,
            tc.tile_pool(name="dram", bufs=2, space="DRAM") as dram,
            tc.tile_pool(name="ps_m", bufs=2, space="PSUM") as ps_m,
            tc.tile_pool(name="ps_t", bufs=2, space="PSUM") as ps_t,
            tc.tile_pool(name="ps_s", bufs=2, space="PSUM") as ps_s,
        ):
            def _copy(eng, dst, src):
                if eng is nc.scalar:
                    eng.copy(dst, src)
                else:
                    eng.tensor_copy(dst, src)
            # ---- dummy warmup collective (absorbs entry barrier) ----
            dz = stat.tile([1, 4], F32)
            if collectives:
                d_in = dram.tile([1, 4], F32, tag="d_in")
                d_out = dram.tile([1, 4], F32, tag="d_out")
                nc.vector.memset(dz, 0.0)
                nc.sync.dma_start(out=d_in[:, :], in_=dz[:, :])
                nc.gpsimd.collective_compute(
                    "AllReduce", mybir.AluOpType.add,
                    replica_groups=[list(range(NCORES))],
                    ins=[d_in[:].opt()], outs=[d_out[:].opt()],
                )
                nc.sync.dma_start(out=dz[:, :], in_=d_out[:, :])
            else:
                nc.vector.memset(dz, 0.0)

            # ---- static SBUF tensors ----
            XT = stat.tile([128, NT, B], F32)        # x^T [(r,i)%128, t, b]
            XB = stat.tile([128, BH, RI], F32)       # x   [b%128, bh, (r,i)]
            WS = stat.tile([128, NT, CO], F32)       # W as lhsT for s-matmul
            WOTB = stat.tile([16, C, RI], BF16)      # W^T bf16 rhs for M-mm
            XTB = stat.tile([128, NT, B], BF16)
            WSB = stat.tile([128, NT, CO], BF16)
            IDB = stat.tile([128, 128], BF16)
            IDF = stat.tile([128, 128], F32)
            nc.sync.dma_start(out=XT, in_=xt_in[:])
            nc.scalar.dma_start(out=XB, in_=xb_in[:])
            nc.sync.dma_start(out=WS, in_=ws_in[:])
            nc.scalar.dma_start(out=WOTB, in_=wot_in[:])
            nc.vector.tensor_copy(XTB[:, :, :], XT[:, :, :])
            nc.vector.tensor_copy(WSB[:, :, :], WS[:, :, :])
            make_identity(nc, IDB[:, :])
            make_identity(nc, IDF[:, :])

            # logits b_ij, layout [p=b%128, (bh, c, r)]
            blog = stat.tile([128, BH, C, RL], F32)

            # s / v in [co, b] layout: two partition tiles (128 + 32 rows)
            sA = stat.tile([128, B], F32)            # co 0..127
            sB = stat.tile([32, B], F32)             # co 128..159
            sAb = stat.tile([128, B], BF16)
            sBb = stat.tile([32, B], BF16)
            vT = stat.tile([16, C, B], BF16)         # v^T [o, c, b] bf16

            def s0_matmul():
                """s0 partials: psum [co,b] f32 (two tiles)."""
                p1 = ps_t.tile([128, B], F32, tag="ep1")
                p2 = ps_t.tile([32, B], F32, tag="ep2")
                for t in range(NT):
                    xcol = XT[:, t, :]
                    nc.tensor.matmul(p1, WS[:, t, 0:128], xcol,
                                     start=(t == 0), stop=(t == NT - 1))
                    nc.tensor.matmul(p2, WS[:, t, 128:160], xcol,
                                     start=(t == 0), stop=(t == NT - 1))
                nc.scalar.copy(sA[:, :], p1[:, :])
                nc.scalar.copy(sB[:, :], p2[:, :])

            def allreduce_s(first, st=None):
                """bounce -> AllReduce -> back to sA/sB [co, b] tiles.

                first: source is sA/sB (s0 path), bounce layout [CO, B].
                else: source is st [16(o), C, B], bounce layout [O, C, B];
                the return DMA scatters (c,o)-major back into sA/sB."""
                if first:
                    b_in = dram.tile([CO, B], F32, tag="arin")
                    b_out = dram.tile([CO, B], F32, tag="arout")
                    if collectives:
                        # liveness tie for the warmup collective (zeros)
                        nc.vector.tensor_add(sA[0:1, 0:4], sA[0:1, 0:4],
                                             dz[0:1, 0:4])
                    nc.sync.dma_start(out=b_in[0:128, :], in_=sA[:, :])
                    nc.scalar.dma_start(out=b_in[128:160, :], in_=sB[:, :])
                else:
                    b_in = dram.tile([O, C, B], F32, tag="arin2")
                    b_out = dram.tile([O, C, B], F32, tag="arout2")
                    nc.sync.dma_start(out=b_in[:, :, :], in_=st[:, :, :])
                if collectives:
                    nc.gpsimd.collective_compute(
                        "AllReduce", mybir.AluOpType.add,
                        replica_groups=[list(range(NCORES))],
                        ins=[b_in[:].opt()], outs=[b_out[:].opt()],
                    )
                else:
                    nc.sync.dma_start(out=b_out[:], in_=b_in[:])
                if first:
                    nc.sync.dma_start(out=sA[:, :], in_=b_out[0:128, :])
                    nc.scalar.dma_start(out=sB[:, :], in_=b_out[128:160, :])
                else:
                    co_b = b_out[:].rearrange("o c b -> c o b")
                    nc.sync.dma_start(out=sA[:, :], in_=co_b[0:8, :, :])
                    nc.scalar.dma_start(out=sB[:, :], in_=co_b[8:10, :, :])

            def squash(scale, last):
                """v = s*|s|/(1+s^2) elementwise on [co,b] tiles (in-place).
                Produces bf16 copies + v^T realign unless last."""
                for s, sb in ((sA, sAb), (sB, sBb)):
                    sq = work.tile(list(s.shape), F32, tag=f"sq{s.shape[0]}")
                    ab = work.tile(list(s.shape), F32, tag=f"ab{s.shape[0]}")
                    sf = s[:, :]
                    if scale != 1.0:
                        nc.scalar.mul(sf, sf, scale)
                    nc.scalar.square(sq[:, :], sf)
                    nc.scalar.sqrt(ab[:, :], sq[:, :])
                    nc.vector.tensor_scalar_add(sq[:, :], sq[:, :], 1.0)
                    nc.vector.reciprocal(sq[:, :], sq[:, :])
                    nc.vector.tensor_mul(ab[:, :], ab[:, :], sq[:, :])
                    nc.vector.tensor_mul(sf, ab[:, :], sf)
                    if not last:
                        nc.vector.tensor_copy(sb[:, :], s[:, :])
                if not last:
                    for c in range(C):
                        src = sAb[c * 16:(c + 1) * 16, :] if c < 8 else \
                            sBb[(c - 8) * 16:(c - 7) * 16, :]
                        qeng = nc.sync if (c % 2 == 0) else nc.scalar
                        qeng.dma_start(out=vT[:, c, :], in_=src)

            def a_phase(first):
                """blog (+)= a;  a_c[b,r] = sum_i x*M, M = v_c @ WoT_c."""
                ar = smp.tile([128, BH, C, RL], F32, tag="ared")
                dst = blog if first else ar
                for c in range(C):
                    for bh in range(BH):
                        lhs = vT[:, c, bh * 128:(bh + 1) * 128]
                        mt = mtp.tile([128, RI], BF16, tag="mtmp")
                        for h in range(3):
                            mp = ps_m.tile([128, H], F32, tag="mpsum")
                            nc.tensor.matmul(mp[:, :], lhs,
                                             WOTB[:, c, h * H:(h + 1) * H],
                                             start=True, stop=True)
                            deng = (nc.scalar, nc.vector,
                                    nc.scalar if c % 2 else nc.vector)[h]
                            _copy(deng, mt[:, h * H:(h + 1) * H], mp[:, :])
                        prod = mtp.tile([128, RI], F32, tag="prod")
                        peng = nc.gpsimd if (c % 2 == 1) else nc.vector
                        peng.tensor_mul(prod[:, :], mt[:, :], XB[:, bh, :])
                        tv = prod[:, :].rearrange("p (r i) -> p r i", i=I)
                        nc.vector.tensor_reduce(dst[:, bh, c, :], tv,
                                                axis=mybir.AxisListType.X,
                                                op=mybir.AluOpType.add)
                if not first:
                    nc.vector.tensor_add(blog[:, :, :, :], blog[:, :, :, :],
                                         ar[:, :, :, :])

            def softmax_en():
                """en = softmax_c(blog) in bf16."""
                # shift by max over c (in place: softmax-invariant, and the
                # shift persists harmlessly across iterations)
                mx = smp.tile([128, BH, RL], F32, tag="mx")
                bv = blog[:, :, :, :].rearrange("p bh c r -> p bh r c")
                nc.vector.tensor_reduce(mx[:, :, :], bv,
                                        axis=mybir.AxisListType.X,
                                        op=mybir.AluOpType.max)
                mxb = _insert_bcast(mx[:, :, :], 2, C)
                nc.vector.tensor_sub(blog[:, :, :, :], blog[:, :, :, :], mxb)
                e = smp.tile([128, BH, C, RL], F32, tag="e")
                nc.scalar.activation(e[:, :, :, :], blog[:, :, :, :],
                                     mybir.ActivationFunctionType.Exp)
                z = smp.tile([128, BH, RL], F32, tag="z")
                ev = e[:, :, :, :].rearrange("p bh c r -> p bh r c")
                nc.vector.tensor_reduce(z[:, :, :], ev,
                                        axis=mybir.AxisListType.X,
                                        op=mybir.AluOpType.add)
                nc.vector.reciprocal(z[:, :, :], z[:, :, :])
                en = smp.tile([128, BH, C, RL], BF16, tag="en")
                zb = _insert_bcast(z[:, :, :], 2, C)
                nc.vector.tensor_mul(en[:, :, :, :], e[:, :, :, :], zb)
                return en

            def s_phase(en, last):
                """en -> enT (PE) -> replicate (DMA) -> y -> s psum -> st."""
                st = smp.tile([16, C, B], F32, tag="st")
                for c in range(C):
                    ep1 = ps_t.tile([128, B], F32, tag="ep1")
                    ep2 = ps_t.tile([32, B], F32, tag="ep2")
                    for bh in range(BH):
                        cols = slice(bh * 128, (bh + 1) * 128)
                        nc.tensor.matmul(ep1[:, cols], en[:, bh, c, 0:128],
                                         IDB[:, :], start=True, stop=True)
                        nc.tensor.matmul(ep2[0:16, cols], en[:, bh, c, 128:RL],
                                         IDB[:, :], start=True, stop=True)
                    et1 = entp.tile([128, B], BF16, tag="et1")
                    et2 = entp.tile([16, B], BF16, tag="et2")
                    nc.scalar.copy(et1[:, :], ep1[:, :])
                    nc.scalar.copy(et2[:, :], ep2[0:16, :])
                    etr = ytp.tile([128, NT, B], BF16, tag="etr")
                    for t in range(NT):
                        if t < 8:
                            base = et1[16 * t:16 * t + 16, :]
                        else:
                            base = et2[0:16, :]
                        src = _insert_bcast(base, 1, I)
                        qeng = nc.sync if (t % 2 == 0) else nc.scalar
                        qeng.dma_start(out=etr[:, t, :], in_=src)
                    sp = ps_s.tile([16, B], F32, tag="spc")
                    if last:
                        ytc = ytp.tile([128, NT, B], BF16, tag="ytcb")
                        meng = nc.gpsimd if (c % 3 == 2) else nc.vector
                        meng.tensor_mul(ytc[:, :, :], etr[:, :, :],
                                        XTB[:, :, :])
                        for t in range(NT):
                            nc.tensor.matmul(sp, WSB[:, t, c * 16:(c + 1) * 16],
                                             ytc[:, t, :],
                                             start=(t == 0), stop=(t == NT - 1))
                    else:
                        ytc = ytp.tile([128, NT, B], F32, tag="ytcf")
                        meng = nc.gpsimd if (c % 3 == 2) else nc.vector
                        meng.tensor_mul(ytc[:, :, :], etr[:, :, :],
                                        XT[:, :, :])
                        for t in range(NT):
                            nc.tensor.matmul(sp, WS[:, t, c * 16:(c + 1) * 16],
                                             ytc[:, t, :],
                                             start=(t == 0), stop=(t == NT - 1))
                    nc.scalar.copy(st[:, c, :], sp[:, :])
                return st

            def emit_output():
                """v [co,b] -> out [b, co] via PE transpose."""
                ob = work.tile([128, BH, CO], F32, tag="ob")
                for bh in range(BH):
                    po = ps_m.tile([128, H], F32, tag="mpsum")
                    cols = slice(bh * 128, (bh + 1) * 128)
                    nc.tensor.matmul(po[:, 0:128], sA[:, cols], IDF[:, :],
                                     start=True, stop=True)
                    nc.tensor.matmul(po[:, 128:160], sB[:, cols],
                                     IDF[0:32, 0:32], start=True, stop=True)
                    nc.scalar.copy(ob[:, bh, :], po[:, 0:160])
                dst = out[:].rearrange("(bh p) co -> p bh co", p=128)
                nc.sync.dma_start(out=dst, in_=ob[:, :, :])

            # ---------------- routing ----------------
            s0_matmul()
            allreduce_s(first=True)
            squash(0.1, last=(n_iters == 1))
            for it in range(1, n_iters):
                last = (it == n_iters - 1)
                a_phase(first=(it == 1))
                en = softmax_en()
                st = s_phase(en, last)
                allreduce_s(first=False, st=st)
                squash(1.0, last=last)
            emit_output()

    nc.compile()
    return nc


def prep_inputs(x: np.ndarray, W: np.ndarray):
    """Host-side layout prep. Returns per-core input dicts."""
    W = W[0]  # [R, C, O, I]
    in_maps = []
    for k in range(NCORES):
        rs = slice(k * RL, (k + 1) * RL)
        xk = np.ascontiguousarray(x[:, rs, :])      # [B, RL, I]
        wk = np.ascontiguousarray(W[rs])            # [RL, C, O, I]
        xt = np.transpose(xk, (1, 2, 0)).reshape(NT, 128, B)
        xt = np.transpose(xt, (1, 0, 2))            # [128, NT, B]
        xb = xk.reshape(BH, 128, RI)
        xb = np.transpose(xb, (1, 0, 2))            # [128, BH, RI]
        # ws[p, t, c*16+o] = W[16t + p//8, c, o, p%8]
        wsk = np.transpose(wk.reshape(NT, 16, C, O, I), (0, 1, 4, 2, 3))
        wsk = wsk.reshape(NT, 128, CO)
        wsk = np.transpose(wsk, (1, 0, 2))          # [128, NT, CO]
        # wot[o, c, r*8+i] = W[r, c, o, i]
        wotk = np.transpose(wk, (2, 1, 0, 3)).reshape(O, C, RI)
        f32 = np.float32
        in_maps.append({
            "xt": np.ascontiguousarray(xt).astype(f32),
            "xb": np.ascontiguousarray(xb).astype(f32),
            "ws": np.ascontiguousarray(wsk).astype(f32),
            "wot": np.ascontiguousarray(wotk).astype(ml_dtypes.bfloat16),
        })
    return in_maps


_CACHE = {}


def _get_nc(n_iters: int):
    if n_iters not in _CACHE:
        _CACHE[n_iters] = build_kernel(n_iters)
    return _CACHE[n_iters]


def kernel(x, W, num_iterations, _trace=False):
    n = int(num_iterations)
    assert n >= 1
    nc = _get_nc(n)
    in_maps = prep_inputs(np.asarray(x, dtype=np.float32),
                          np.asarray(W, dtype=np.float32))
    res = run_bass_kernel_spmd(nc, in_maps, list(range(NCORES)),
                               trace=_trace)
    v = res.results[0]["out"].reshape(B, C, O, 1).astype(np.float32)
    kernel.last_results = res
    return v
